# revision 12
# baseline (speedup 1.0000x reference)
"""LSTM sequence classifier on 8 Trainium2 NeuronCores.

Data-parallel over batch (512 seqs/core), lengths sorted+dealt so all
cores share one length multiset; per-step work shrinks to the active
prefix. Per core the gate matmul grid is retiled for PE efficiency:

- 10 M-units instead of 12: full 128-row units [i0 f0 o0 i1 f1 o1 g0 g1]
  plus two col-paired tail units P8=(i2|f2) and P9=(o2|g2) holding the
  44-row gate tails at output partitions 0-43 / 64-107.
- K-tails (x dims 256-299, h dims 256-299) issue as 44/45-row matmuls on
  row-groups 0 / 64 (alternating per unit) so consecutive tail matmuls
  co-run in the PE array. Sub-2 operand data is duplicated at partitions
  0-43 and 64-107 (embedding table layout / DVE dup write).
- Biases ride a constant-1 row in the x-tail weights (emb col 300/364).
- One static 30-slot PSUM tensor (3 rotation groups x 10 units x 128),
  drains fused across units and seg-pairs (4 ACTs per seg-pair).
- Gates drain to bf16; c stays fp32; retire muls run on GpSimd (idle
  after gathers, off the recurrence chain).
- Next-step x-matmuls are emitted ahead of the cell-update chain so the
  PE never idles on the recurrence.
"""
import sys

sys.path.insert(0, "/opt/trn_rl_repo")

import numpy as np
import ml_dtypes

import concourse.bass as bass
import concourse.tile as tile
from concourse import bacc, mybir
from concourse.bass_utils import run_bass_kernel_spmd

V, E, H, T, B = 30000, 300, 300, 22, 4096
NCORES = 8
EP = 384          # padded embedding row; sub2 dup at cols 320-363, bias 300/364
W = 128           # PSUM segment width
NG = 2            # PSUM rotation groups (12 slots = 3 banks each, bank-aligned)
F32 = mybir.dt.float32
BF16 = mybir.dt.bfloat16
I16 = mybir.dt.int16
AF = mybir.ActivationFunctionType

# slot u -> (kind, sub) ; kind: gate letter or pair
SLOTS = [("i", 0), ("f", 0), ("o", 0), ("i", 1), ("f", 1), ("o", 1),
         ("P8", 2), ("g", 0), ("g", 1), ("P9", 2)]
PAIR_HALVES = {"P8": ("i", "f"), "P9": ("o", "g")}
GBASE = {"i": 0, "f": 300, "g": 600, "o": 900}
TAILBASE = [0, 64, 0, 64, 0, 64, 0, 64, 0, 64]

_patched = False


def _patch_tile_drain():
    """walrus CTRL (Drain) supports fewer sem waits than Tile attaches at
    the kernel tail; spread them across single-wait SP NOPs instead."""
    global _patched
    if _patched:
        return
    _patched = True
    import concourse.tile as tile_mod
    from concourse.vector_clock import ScopedClock

    def _drain_and_barrier(self, tick_clock, wait_clock):
        nc = self.nc
        probe = nc.sync.nop(nofuse=True)
        wait_clock.add_sem_waits(
            probe.ins, ScopedClock({None: tick_clock.global_clock}))
        si = probe.ins.sync_info
        waits = list(si.on_wait) if si is not None else []
        upds = list(si.on_update) if si is not None else []
        probe.ins.sync_info = mybir.SyncInfo(on_wait=waits[:1], on_update=upds)
        for w in waits[1:]:
            n2 = nc.sync.nop(nofuse=True)
            n2.ins.sync_info = mybir.SyncInfo(on_wait=[w], on_update=[])
        nc.sync.drain()
        nc.all_engine_barrier()
        popped = nc._tile_sem_poison_stack.pop()
        assert popped is self._sem_poison
        nc.clear_and_free_semaphores(list(self.sems.allocated().values()))
        nc.all_engine_barrier()

    tile_mod.TileContext._drain_and_barrier = _drain_and_barrier


def _schedule(cap_len):
    """Deal batches to cores so every core has the same length multiset."""
    q = np.zeros(T + 1, np.int64)
    orders = [[] for _ in range(NCORES)]
    for l in range(T, 0, -1):
        idxs = np.nonzero(cap_len == l)[0]
        k = len(idxs)
        ql = -(-k // NCORES)
        q[l] = ql
        for c in range(NCORES):
            part = idxs[c::NCORES]
            orders[c].extend(int(x) for x in part)
            orders[c].extend([-1] * (ql - len(part)))
    n_t = [int(q[t + 1:].sum()) for t in range(T)]
    return orders, n_t


def _chunk_groups(n_t):
    """Group whole steps into gather chunks (first chunks small)."""
    groups = [[0], [1]]
    t = 2
    while t < T:
        g, s = [], 0
        while t < T and (s < 1100 or not g):
            g.append(t)
            s += n_t[t]
            t += 1
        groups.append(g)
    return groups


def _build_program(n_t, chunks):
    """chunks: list of dicts {steps, goff, clenp, a: {t: local offset}}"""
    Q = n_t[0]
    QR = Q
    NIDX = sum(c["clenp"] for c in chunks)
    chunk_of_step = {}
    for ci, c in enumerate(chunks):
        for t in c["steps"]:
            chunk_of_step[t] = ci

    nc = bacc.Bacc("TRN2", target_bir_lowering=False, debug=False,
                   num_swdge_queues=2)
    emb_d = nc.dram_tensor("emb", [V, EP], BF16, kind="ExternalInput")
    idx_d = nc.dram_tensor("idx", [128, NIDX // 16], I16, kind="ExternalInput")
    wall_d = nc.dram_tensor("wall", [128, 6, 1280], BF16, kind="ExternalInput")
    vt_d = nc.dram_tensor("vt", [128, 3, 2], BF16, kind="ExternalInput")
    bc_d = nc.dram_tensor("bc", [2, 1], F32, kind="ExternalInput")
    out_d = nc.dram_tensor("out", [2, Q], F32, kind="ExternalOutput")

    with tile.TileContext(nc) as tc:
        with tc.tile_pool(name="sb", bufs=1) as sp:
            # ---- constant loads: idx first (gathers gate on it) ----
            idx_sb = sp.tile([128, NIDX // 16], I16, tag="idx")
            nc.sync.dma_start(out=idx_sb[:], in_=idx_d[:])
            wall_sb = sp.tile([128, 6, 1280], BF16, tag="wall")
            # x-weights (kg 0-2) first, split across both HWDGE queues
            nc.scalar.dma_start(out=wall_sb[:, 0, :], in_=wall_d[:, 0, :])
            nc.sync.dma_start(out=wall_sb[:, 1, :], in_=wall_d[:, 1, :])
            nc.scalar.dma_start(out=wall_sb[:, 2, :], in_=wall_d[:, 2, :])
            nc.sync.dma_start(out=wall_sb[:, 3, :], in_=wall_d[:, 3, :])
            nc.scalar.dma_start(out=wall_sb[:, 4, :], in_=wall_d[:, 4, :])
            nc.sync.dma_start(out=wall_sb[:, 5, :], in_=wall_d[:, 5, :])
            vt_sb = sp.tile([128, 3, 2], BF16, tag="vt")
            nc.sync.dma_start(out=vt_sb[:], in_=vt_d[:])
            bc_sb = sp.tile([2, 1], F32, tag="bc")
            nc.sync.dma_start(out=bc_sb[:], in_=bc_d[:])

            # ---- gathers (all emitted up-front on gpsimd) ----
            xts = []
            for ci, c in enumerate(chunks):
                xt = sp.tile([128, 3, c["clenp"]], BF16, tag=f"xt{ci}")
                nc.gpsimd.dma_gather(
                    out_ap=xt[:], in_ap=emb_d[:],
                    idxs_ap=idx_sb[:, c["goff"] // 16:
                                   (c["goff"] + c["clenp"]) // 16],
                    num_idxs=c["clenp"], num_idxs_reg=c["clenp"],
                    elem_size=EP, transpose=True, single_packet=False,
                    queue_num=ci % 2)
                xts.append(xt)

            # ---- state ----
            ps = nc.alloc_psum_tensor("PS", [128, NG, 12, W], F32)
            ph = nc.alloc_psum_tensor("PH", [2, 512], F32)
            # slots 0-9 as documented; 10 = f2 shifted to 0:44, 11 = g2
            gAll = sp.tile([128, 12, QR], BF16, tag="gAll")
            cT = sp.tile([128, 3, QR], F32, tag="cT")
            th = sp.tile([128, 3, QR], BF16, tag="th")
            hT = [sp.tile([128, 3, QR], BF16, tag=f"h{i}", name=f"h{i}")
                  for i in range(2)]
            lastT = sp.tile([128, 3, QR], BF16, tag="lastT")
            out_sb = sp.tile([2, QR], F32, tag="out_sb")

            # start=True clears has_written for the WHOLE bank: only the
            # first MM per bank (units 0/4/8, first k-group) carries start,
            # only the last per bank (units 3/7/9, last k-group) carries stop.
            def mm_unit(slot, g, kg, rhs, w, start, stop):
                """one matmul of k-group kg for unit `slot` into PS group g."""
                start = start and slot in (0, 4, 8)
                stop = stop and slot in (3, 7, 9)
                pslot = ps[:, g, slot, :w]
                wcol = wall_sb[:, kg, 128 * slot:128 * (slot + 1)]
                if kg in (0, 1, 3, 4):
                    nc.tensor.matmul(pslot, wcol, rhs, start=start, stop=stop)
                else:
                    tb = TAILBASE[slot]
                    nk = 45 if kg == 2 else 44
                    nc.tensor.matmul(
                        pslot, wcol[tb:tb + nk, :], rhs[tb:tb + nk, :],
                        start=start, stop=stop)

            def seg_list(t):
                n = n_t[t]
                ci = chunk_of_step[t]
                a0 = chunks[ci]["a"][t]
                segs = []
                o = 0
                while o < n:
                    w = min(W, n - o)
                    segs.append((ci, a0 + o, o, w))
                    o += w
                return segs

            seg_group = {}   # (t, j) -> group

            def emit_x(t, j, segs):
                ci, a, o, w = segs[j]
                g = seg_group[(t, j)]
                for u in range(10):
                    for kg in (0, 1, 2):
                        rhs = xts[ci][:, kg, a:a + w]
                        mm_unit(u, g, kg, rhs, w,
                                start=(kg == 0), stop=False)

            def emit_h(t, j, segs, hIn):
                ci, a, o, w = segs[j]
                g = seg_group[(t, j)]
                for u in range(10):
                    for kg in (3, 4, 5):
                        rhs = hIn[:, kg - 3, o:o + w]
                        mm_unit(u, g, kg, rhs, w, start=False, stop=(kg == 5))

            def emit_x_stop(t, j, segs):
                # t == 0: x-only accumulation needs its own stop marker
                ci, a, o, w = segs[j]
                g = seg_group[(t, j)]
                for u in range(10):
                    for kg in (0, 1, 2):
                        rhs = xts[ci][:, kg, a:a + w]
                        mm_unit(u, g, kg, rhs, w,
                                start=(kg == 0), stop=(kg == 2))

            def drain(t, jlist, segs):
                fuse = (len(jlist) == 2
                        and seg_group[(t, jlist[0])] == 0
                        and seg_group[(t, jlist[1])] == 1
                        and segs[jlist[0]][3] == segs[jlist[1]][3])
                if fuse:
                    w, o = segs[jlist[0]][3], segs[jlist[0]][2]
                    nc.scalar.activation(
                        gAll[:, 0:7, o:o + 2 * w].rearrange(
                            "p u (s w) -> p s u w", s=2),
                        ps[:, 0:2, 0:7, :w], AF.Sigmoid)
                    nc.scalar.activation(
                        gAll[:, 7:9, o:o + 2 * w].rearrange(
                            "p u (s w) -> p s u w", s=2),
                        ps[:, 0:2, 7:9, :w], AF.Tanh)
                    nc.scalar.activation(
                        gAll[0:44, 9, o:o + 2 * w].rearrange(
                            "p (s w) -> p s w", s=2),
                        ps[0:44, 0:2, 9, :w], AF.Sigmoid)
                    nc.scalar.activation(
                        gAll[0:44, 10, o:o + 2 * w].rearrange(
                            "p (s w) -> p s w", s=2),
                        ps[64:108, 0:2, 6, :w], AF.Sigmoid)
                    nc.scalar.activation(
                        gAll[0:44, 11, o:o + 2 * w].rearrange(
                            "p (s w) -> p s w", s=2),
                        ps[64:108, 0:2, 9, :w], AF.Tanh)
                    return
                for j in jlist:
                    g, w, o = seg_group[(t, j)], segs[j][3], segs[j][2]
                    nc.scalar.activation(gAll[:, 0:7, o:o + w],
                                         ps[:, g, 0:7, :w], AF.Sigmoid)
                    nc.scalar.activation(gAll[:, 7:9, o:o + w],
                                         ps[:, g, 7:9, :w], AF.Tanh)
                    nc.scalar.activation(gAll[0:44, 9, o:o + w],
                                         ps[0:44, g, 9, :w], AF.Sigmoid)
                    nc.scalar.activation(gAll[0:44, 10, o:o + w],
                                         ps[64:108, g, 6, :w], AF.Sigmoid)
                    nc.scalar.activation(gAll[0:44, 11, o:o + w],
                                         ps[64:108, g, 9, :w], AF.Tanh)

            # gate slot views per sub: (i, f, o, g) APs for cols :n
            def gate_aps(s, n):
                if s < 2:
                    return (gAll[:, 3 * s + 0, :n], gAll[:, 3 * s + 1, :n],
                            gAll[:, 3 * s + 2, :n], gAll[:, 7 + s, :n])
                return (gAll[0:44, 6, :n], gAll[0:44, 10, :n],
                        gAll[0:44, 9, :n], gAll[0:44, 11, :n])

            hprev = None
            par = 0
            for t in range(T):
                n = n_t[t]
                if n == 0:
                    continue
                segs = seg_list(t)
                ns = len(segs)
                cap = n_t[t + 1] if t < T - 1 else 0
                hOut = hT[(t + 1) % 2]

                # group parity: multi-seg steps restart at group 0 so seg
                # pairs are always (0,1) and drains fuse; 1-seg steps
                # alternate so the next step's x isn't blocked on this drain
                if ns >= 2:
                    for j in range(ns):
                        seg_group[(t, j)] = j % 2
                    par = ns % 2
                else:
                    seg_group[(t, 0)] = par
                    par ^= 1

                # x matmuls (up to NG segs ahead of drains)
                emitx = emit_x if t > 0 else emit_x_stop
                for j in range(min(NG, ns)):
                    emitx(t, j, segs)
                # h matmuls + drains per seg pair; late x after drains free groups
                jp = [(j, j + 1) if j + 1 < ns else (j,)
                      for j in range(0, ns, 2)]
                for pi, pr in enumerate(jp):
                    if t > 0:
                        for j in pr:
                            emit_h(t, j, segs, hprev)
                    drain(t, list(pr), segs)
                    for j in pr:
                        if j + NG < ns:
                            emitx(t, j + NG, segs)

                # ---- cell update (per sub; sub2 ragged) ----
                def o_slice(s, lo, hi):
                    if s == 2:
                        return gAll[0:44, 9, lo:hi]
                    return gAll[:, 3 * s + 2, lo:hi]

                def psl_of(s):
                    return slice(0, 44) if s == 2 else slice(0, 128)

                def survive_mul(s):
                    psl = psl_of(s)
                    nc.vector.tensor_mul(hOut[psl, s, :cap],
                                         o_slice(s, 0, cap),
                                         th[psl, s, :cap])
                    if s == 2:
                        nc.vector.tensor_mul(hOut[64:108, 2, :cap],
                                             o_slice(2, 0, cap),
                                             th[0:44, 2, :cap])

                # chain pass: tmp/fc/add + tanh per sub; survive-mul of sub
                # s-1 emitted after sub s's adds so tanh latency hides
                for s in range(3):
                    i_ap, f_ap, o_ap, g_ap = gate_aps(s, n)
                    psl = psl_of(s)
                    if t == 0:
                        nc.vector.tensor_mul(cT[psl, s, :n], i_ap, g_ap)
                    else:
                        # tmp = i*g  (in-place into i slot)
                        nc.vector.tensor_mul(i_ap, i_ap, g_ap)
                        nc.vector.tensor_mul(cT[psl, s, :n], f_ap,
                                             cT[psl, s, :n])
                        nc.vector.tensor_add(cT[psl, s, :n], cT[psl, s, :n],
                                             i_ap)
                    nc.scalar.activation(th[psl, s, :n], cT[psl, s, :n],
                                         AF.Tanh)
                    if cap > 0 and s > 0:
                        survive_mul(s - 1)
                if cap > 0:
                    survive_mul(2)
                if cap < n:
                    # retire: off-chain, run on gpsimd
                    for s in range(3):
                        nc.gpsimd.tensor_mul(lastT[psl_of(s), s, cap:n],
                                             o_slice(s, cap, n),
                                             th[psl_of(s), s, cap:n])
                hprev = hOut

            # ---- head: logits^T = Whead @ lastT + b ----
            col = 0
            while col < Q:
                w = min(512, Q - col)
                nc.tensor.matmul(ph[:, :w], vt_sb[:, 0, :],
                                 lastT[:, 0, col:col + w],
                                 start=True, stop=False)
                nc.tensor.matmul(ph[:, :w], vt_sb[:, 1, :],
                                 lastT[:, 1, col:col + w],
                                 start=False, stop=False)
                nc.tensor.matmul(ph[:, :w], vt_sb[0:44, 2, :],
                                 lastT[0:44, 2, col:col + w],
                                 start=False, stop=True)
                nc.scalar.activation(out_sb[:, col:col + w], ph[:, :w],
                                     AF.Identity, bias=bc_sb[:, 0:1],
                                     scale=1.0)
                col += w
            nc.sync.dma_start(out=out_d[:], in_=out_sb[:, :Q])

    nc.compile()
    return nc


def _pack_weights(W_ih, W_hh, b_ih, b_hh):
    btot = b_ih + b_hh
    wall = np.zeros((128, 6, 1280), np.float32)

    def outrow(u, c):
        kind, s = SLOTS[u]
        if kind in PAIR_HALVES:
            ka, kb = PAIR_HALVES[kind]
            if 0 <= c < 44:
                return GBASE[ka] + 256 + c
            if 64 <= c < 108:
                return GBASE[kb] + 256 + (c - 64)
            return None
        r = GBASE[kind] + 128 * s + c
        return r if (r - GBASE[kind]) < 300 else None

    for u in range(10):
        tb = TAILBASE[u]
        for c in range(128):
            r = outrow(u, c)
            if r is None:
                continue
            col = 128 * u + c
            wall[:, 0, col] = W_ih[r, 0:128]
            wall[:, 1, col] = W_ih[r, 128:256]
            wall[tb:tb + 44, 2, col] = W_ih[r, 256:300]
            wall[tb + 44, 2, col] = btot[r]
            wall[:, 3, col] = W_hh[r, 0:128]
            wall[:, 4, col] = W_hh[r, 128:256]
            wall[tb:tb + 44, 5, col] = W_hh[r, 256:300]
    return wall.astype(ml_dtypes.bfloat16)


def _prep_and_run(inputs, trace=False):
    _patch_tile_drain()
    cap = np.asarray(inputs["cap"]).astype(np.int64)
    cap_len = np.asarray(inputs["cap_len"]).astype(np.int64)
    embed = np.asarray(inputs["embed"], np.float32)
    W_ih = np.asarray(inputs["W_ih"], np.float32)
    W_hh = np.asarray(inputs["W_hh"], np.float32)
    b_ih = np.asarray(inputs["b_ih"], np.float32)
    b_hh = np.asarray(inputs["b_hh"], np.float32)
    v_wn = np.asarray(inputs["v_wn"], np.float32)
    g_wn = np.asarray(inputs["g_wn"], np.float32)
    b_cls = np.asarray(inputs["b_cls"], np.float32)

    orders, n_t = _schedule(cap_len)
    Q = n_t[0]

    # chunks: groups of whole steps, each padded to x16 tokens
    groups = _chunk_groups(n_t)
    chunks = []
    goff = 0
    for g in groups:
        clen = sum(n_t[t] for t in g)
        clenp = -(-clen // 128) * 128
        a = {}
        o = 0
        for t in g:
            a[t] = o
            o += n_t[t]
        chunks.append({"steps": g, "goff": goff, "clenp": clenp, "a": a})
        goff += clenp
    NIDX = goff

    # per-core token streams
    idx_maps = []
    for c in range(NCORES):
        order = np.asarray(orders[c], np.int64)
        toks = np.zeros(NIDX, np.int16)
        for ch in chunks:
            for t in ch["steps"]:
                n = n_t[t]
                sel = order[:n]
                tk = np.where(sel >= 0, cap[np.clip(sel, 0, None), t], 0)
                toks[ch["goff"] + ch["a"][t]:
                     ch["goff"] + ch["a"][t] + n] = tk.astype(np.int16)
        packed = np.tile(toks.reshape(NIDX // 16, 16).T, (8, 1)).copy()
        idx_maps.append(packed)

    # embedding table: dup sub2 tail + bias-const columns
    emb_pad = np.zeros((V, EP), np.float32)
    emb_pad[:, :E] = embed
    emb_pad[:, 300] = 1.0
    emb_pad[:, 320:364] = embed[:, 256:300]
    emb_pad[:, 364] = 1.0
    emb_pad = emb_pad.astype(ml_dtypes.bfloat16)

    wall_np = _pack_weights(W_ih, W_hh, b_ih, b_hh)

    # head weights (weight-norm applied host-side, like the host bias fold)
    Whead = (g_wn[:, None] * v_wn /
             np.linalg.norm(v_wn, axis=1, keepdims=True))  # [2, 300]
    vt_np = np.zeros((128, 3, 2), np.float32)
    for k in range(3):
        d = min(128, 300 - 128 * k)
        vt_np[0:d, k, :] = Whead[:, 128 * k:128 * k + d].T
    vt_np = vt_np.astype(ml_dtypes.bfloat16)
    bc_np = np.ascontiguousarray(b_cls.reshape(2, 1)).astype(np.float32)

    nc = _build_program(n_t, chunks)

    in_maps = []
    for c in range(NCORES):
        in_maps.append({"emb": emb_pad, "idx": idx_maps[c], "wall": wall_np,
                        "vt": vt_np, "bc": bc_np})
    res = run_bass_kernel_spmd(nc, in_maps, list(range(NCORES)), trace=trace)

    out = np.zeros((B, 2), np.float32)
    for c in range(NCORES):
        logitsT = res.results[c]["out"]  # [2, Q]
        order = orders[c]
        for pos, gi in enumerate(order):
            if gi >= 0:
                out[gi] = logitsT[:, pos]
    return out, res


def kernel(**inputs):
    out, _ = _prep_and_run(inputs, trace=False)
    return out
